# revision 43
# baseline (speedup 1.0000x reference)
"""Trainium2 Bass kernel for CanonCausalMultiheadAttn.

Sharding: tensor-parallel over heads across 8 cores (2 q-heads + 1 kv-head
per core), both batches replicated. Two head-split AllToAlls exchange
attention outputs so each core owns one (batch, seq-slice) of the final
output projection; the first overlaps the second half of attention and the
second overlaps the h=0 partial pass of the output projection.

Per-core pipeline (shapes hardcoded for B=2, S=2048, D=2048):
  QKV proj (bf16 matmul, weights SBUF-resident) -> canon conv via halo'd
  raw buffer (DVE, bf16) -> qk rmsnorm rstd via PE column-sum matmuls and
  fast-approx reciprocal -> RoPE (DVE bf16; norm-weight & 1/sqrt(dh)
  folded into host tables; q AND k rstd folded in via K=1 broadcast
  matmuls whose emission is software-pipelined into later PE-dense
  phases) -> causal attention with scores in [Sk, Sq] layout, two
  Sk-blocks paired per [128,1024] PSUM tile so one exp covers both; the
  causal mask and the fully-masked wedge are added on the PE itself
  (maskT.T @ I accumulation) so exp depends only on the PE; PV computed
  transposed (V stationary) directly in [dh, Sq]; softmax denominator via
  ones-column matmuls; normalize tail software-pipelined into the next
  block -> AllToAll x2 (head-split) -> output projection in two passes
  (h=0 partials to SBUF during the second AllToAll, then h=1 + combine).
"""
import sys

sys.path.insert(0, '/opt/trn_rl_repo')

import numpy as np
import ml_dtypes

import concourse.bass as bass
import concourse.mybir as mybir
import concourse.tile as tile
from concourse import bacc
from concourse.bass_utils import run_bass_kernel_spmd

F32 = mybir.dt.float32
F32R = mybir.dt.float32r
BF16 = mybir.dt.bfloat16
AF = mybir.ActivationFunctionType
ALU = mybir.AluOpType

B, S, D = 2, 2048, 2048
NH, NKV, DH = 16, 8, 128
K_CONV = 4
EPS = 1e-6
SCALE = 1.0 / float(np.sqrt(DH))
NEG = -1e9
N_CORES = 8
NCB = S // 512          # 512-token chunks per batch
N_SKB = S // 128        # Sk blocks per batch


def _build():
    nc = bacc.Bacc("TRN2", target_bir_lowering=False, debug=False,
                   num_devices=N_CORES)

    hsT = nc.dram_tensor("hsT", [D, B * S], BF16, kind="ExternalInput")
    wT = nc.dram_tensor("wT", [D, 512], BF16, kind="ExternalInput")
    woT = nc.dram_tensor("woT", [D, D], BF16, kind="ExternalInput")
    cw = nc.dram_tensor("cw", [512, K_CONV], F32, kind="ExternalInput")
    ropeAq = nc.dram_tensor("ropeAq", [DH, S], BF16, kind="ExternalInput")
    ropeBq = nc.dram_tensor("ropeBq", [DH, S], BF16, kind="ExternalInput")
    ropeAk = nc.dram_tensor("ropeAk", [DH, S], BF16, kind="ExternalInput")
    ropeBk = nc.dram_tensor("ropeBk", [DH, S], BF16, kind="ExternalInput")
    maskTb = nc.dram_tensor("maskTb", [128, 128], BF16, kind="ExternalInput")
    idb = nc.dram_tensor("idb", [128, 128], BF16, kind="ExternalInput")
    out = nc.dram_tensor("out", [512, D], F32, kind="ExternalOutput")

    with tile.TileContext(nc) as tc:
        with tc.tile_pool(name="const", bufs=1) as cpool, \
             tc.tile_pool(name="persist", bufs=1) as pers, \
             tc.tile_pool(name="dram", bufs=1, space="DRAM") as dram:

            # QKV weights resident in SBUF: [128, 16 k-blocks x 512]
            wT_sb = cpool.tile([128, 16 * 512], BF16, tag="wTsb")
            wv = wT_sb[:].rearrange("p (k c) -> p k c", c=512)
            for kk in range(4):
                nc.sync.dma_start(
                    wv[:, 4 * kk:4 * (kk + 1), :],
                    wT.ap()[512 * kk:512 * (kk + 1), :]
                    .rearrange("(k p) c -> p k c", p=128))

            # ---- constants (rope tables last: not needed until late) ----
            mask_sb = cpool.tile([128, 128], BF16, tag="mask")
            nc.sync.dma_start(mask_sb[:], maskTb.ap())
            id_sb = cpool.tile([128, 128], BF16, tag="idsb")
            nc.sync.dma_start(id_sb[:], idb.ap())
            cw_sb = []
            for mt in range(4):
                t = cpool.tile([128, K_CONV], F32, tag=f"cw{mt}",
                               name=f"cw{mt}")
                nc.sync.dma_start(t[:], cw.ap()[128 * mt:128 * mt + 128, :])
                cw_sb.append(t)
            ones_col = cpool.tile([128, 1], BF16, tag="oc")
            nc.vector.memset(ones_col[:], 1.0)
            eps_sb = cpool.tile([1, 1], F32, tag="eps")
            nc.vector.memset(eps_sb[:], EPS)
            ones_row = cpool.tile([1, 128], F32, tag="or")
            nc.vector.memset(ones_row[:], 1.0)
            # NEG broadcast: negrow.T @ onesb[:, :w] == NEG everywhere
            negrow = cpool.tile([128, 128], BF16, tag="negrow")
            nc.vector.memset(negrow[:], 0.0)
            nc.vector.memset(negrow[0:1, :], NEG)
            onesb = cpool.tile([128, 512], BF16, tag="onesb")
            nc.vector.memset(onesb[:], 1.0)
            s0_sb = []
            for mt in range(4):
                t = cpool.tile([128, 1], F32, tag=f"s0{mt}", name=f"s0{mt}")
                nc.vector.tensor_scalar_add(t[:], cw_sb[mt][:, 0:1], 1.0)
                s0_sb.append(t)
            ropes = {}
            for nm in ("Aq", "Bq", "Ak", "Bk"):
                ropes[nm] = cpool.tile([DH, S], BF16, tag=f"rope{nm}",
                                       name=f"rope{nm}")

            # persistent per-(b,mt) tiles
            roped = {}   # (b, mt) -> [128, S] bf16 (rstd folded in)
            vT = {}      # b -> [128, N_SKB*128] bf16 (V transposed blocks)

            for b in range(B):
                vT[b] = pers.tile([128, N_SKB * 128], BF16, tag=f"vT{b}",
                                  name=f"vT{b}")
                for mt in range(3):
                    roped[(b, mt)] = pers.tile(
                        [128, S], BF16, tag=f"roped{b}{mt}",
                        name=f"roped{b}{mt}")

            # ============ phase Q: QKV + canon + norm + rope ============
            # attention working tiles live at top level so they never
            # land on recycled phase-Q scratch space (avoids end-of-phase
            # write-after-read stalls)
            atop_ctx = tc.tile_pool(name="atop", bufs=1)
            atop = atop_ctx.__enter__()

            with tc.tile_pool(name="qps", bufs=1, space="PSUM") as qps, \
                 tc.tile_pool(name="spp", bufs=2, space="PSUM") as spp, \
                 tc.tile_pool(name="bwork", bufs=1) as bw:
                cn = {}
                raw_h = {}
                for mt in range(4):
                    cn[mt] = bw.tile([128, S], BF16, tag=f"cn{mt}",
                                     name=f"cn{mt}")
                    raw_h[mt] = bw.tile([128, 516], BF16,
                                        tag=f"rawh{mt}",
                                        name=f"raw_h{mt}")
                psums = [qps.tile([128, 512], F32, tag=f"qk{mt}",
                                  name=f"qk{mt}") for mt in range(4)]
                for b in range(B):
                    if True:
                        for mt in range(4):
                            nc.vector.memset(raw_h[mt][:, 0:4], 0.0)

                        def emit_chunk_mms(n):
                            hs_sb = bw.tile([128, 16 * 512], BF16,
                                            tag="hschunk", bufs=2,
                                            name="hs_sb")
                            hsv = hs_sb[:].rearrange("p (k s) -> p k s",
                                                     s=512)
                            for kh in range(2):
                                nc.sync.dma_start(
                                    hsv[:, 8 * kh:8 * (kh + 1), :],
                                    hsT.ap()[1024 * kh:1024 * (kh + 1),
                                             b * S + 512 * n:
                                             b * S + 512 * (n + 1)]
                                    .rearrange("(k p) s -> p k s", p=128))
                            hvv = hs_sb[:].rearrange("p (k s) -> p k s",
                                                     s=512)
                            for k in range(16):
                                for mt in range(4):
                                    nc.tensor.matmul(
                                        psums[mt][:],
                                        wv[:, k, 128 * mt:128 * (mt + 1)],
                                        hvv[:, k, :],
                                        start=(k == 0), stop=(k == 15))
                            if b == 0 and n == 1:
                                for nm, t in (("Aq", ropeAq), ("Bq", ropeBq),
                                              ("Ak", ropeAk), ("Bk", ropeBk)):
                                    nc.sync.dma_start(ropes[nm][:], t.ap())

                        def emit_raw(n):
                            # drain the QKV PSUM banks on ACT (it has
                            # slack) so the next chunk's matmuls restart
                            # promptly without waiting on the DVE backlog
                            for mt in range(4):
                                rh = raw_h[mt]
                                if n > 0:
                                    nc.vector.tensor_copy(
                                        rh[:, 1:4], rh[:, 513:516])
                                nc.scalar.copy(rh[:, 4:516], psums[mt][:])

                        def emit_canon(n):
                            lo = 512 * n
                            for mt in range(4):
                                rh = raw_h[mt]
                                c = cn[mt]
                                nc.vector.tensor_scalar_mul(
                                    c[:, lo:lo + 512], rh[:, 4:516],
                                    s0_sb[mt][:])
                                for k in range(1, K_CONV):
                                    nc.vector.scalar_tensor_tensor(
                                        c[:, lo:lo + 512],
                                        rh[:, 4 - k:516 - k],
                                        cw_sb[mt][:, k:k + 1],
                                        c[:, lo:lo + 512],
                                        ALU.mult, ALU.add)
                            # squares for rmsnorm (q0, q1, k)
                            for mt in range(3):
                                sq = bw.tile([128, 512], BF16, tag="sqr",
                                             bufs=3, name="sq")
                                nc.vector.tensor_mul(
                                    sq[:], cn[mt][:, lo:lo + 512],
                                    cn[mt][:, lo:lo + 512])
                                sqs[(n, mt)] = sq

                        def emit_norm(n):
                            for mt in range(3):
                                sp = spp.tile([1, 512], F32, tag="ssq")
                                nc.tensor.matmul(sp[:], ones_col[:],
                                                 sqs.pop((n, mt))[:],
                                                 start=True, stop=True)
                                srt = bw.tile([1, 512], F32, tag="srt",
                                              bufs=2, name="srt")
                                nc.scalar.activation(srt[:], sp[:], AF.Sqrt,
                                                     bias=eps_sb[:],
                                                     scale=1.0 / DH)
                                rn = bw.tile([1, 512], F32, tag=f"rn{mt}",
                                             bufs=3, name=f"rn{mt}")
                                nc.vector.reciprocal_approx_fast(
                                    rn[:], srt[:])
                                rns[(n, mt)] = rn
                            # V transpose blocks for this chunk
                            for t in range(4):
                                i = 4 * n + t
                                nc.sync.dma_start_transpose(
                                    vT[b][:, 128 * i:128 * (i + 1)],
                                    cn[3][:, 128 * i:128 * (i + 1)])

                        def emit_bc(n):
                            # rstd broadcast for chunk n via gpsimd
                            # partition-broadcast (no PE/ACT involvement)
                            for mt in range(3):
                                rnb = bw.tile([1, 512], BF16, tag="rnb",
                                              bufs=3, name="rnb")
                                nc.vector.tensor_copy(
                                    rnb[:], rns.pop((n, mt))[:])
                                bcb = bw.tile([128, 512], BF16, tag="bcs",
                                              bufs=4, name="bcs")
                                nc.gpsimd.partition_broadcast(
                                    bcb[:], rnb[:])
                                bcb_store[(mt, n)] = bcb

                        def emit_rope1(n):
                            # chunk-wise rope + in-place rstd scale so
                            # roped is complete when the pipeline drains
                            lo = 512 * n
                            for mt in (2, 0, 1):
                                is_q = mt < 2
                                x = cn[mt]
                                A_ = ropes["Aq"] if is_q else ropes["Ak"]
                                B_ = ropes["Bq"] if is_q else ropes["Bk"]
                                sh = bw.tile([128, 512], BF16, tag="shift",
                                             bufs=3, name="sh")
                                nc.gpsimd.dma_start(sh[0:64, :],
                                                    x[64:128, lo:lo + 512])
                                nc.gpsimd.dma_start(sh[64:128, :],
                                                    x[0:64, lo:lo + 512])
                                nc.vector.tensor_mul(sh[:], sh[:],
                                                     B_[:, lo:lo + 512])
                                ro = roped[(b, mt)]
                                nc.vector.tensor_mul(ro[:, lo:lo + 512],
                                                     x[:, lo:lo + 512],
                                                     A_[:, lo:lo + 512])
                                nc.vector.tensor_add(ro[:, lo:lo + 512],
                                                     ro[:, lo:lo + 512],
                                                     sh[:])
                                nc.vector.tensor_mul(
                                    ro[:, lo:lo + 512],
                                    ro[:, lo:lo + 512],
                                    bcb_store.pop((mt, n))[:])

                        sqs = {}
                        rns = {}
                        bcb_store = {}
                        for n in range(NCB + 2):
                            if n < NCB:
                                emit_chunk_mms(n)
                            if 1 <= n <= NCB:
                                emit_norm(n - 1)
                            if n >= 2:
                                emit_bc(n - 2)
                            if n < NCB:
                                emit_raw(n)
                                emit_canon(n)
                            if n >= 2:
                                emit_rope1(n - 2)

            # ============ attention + head-split all-to-all ============
            wpool_ctx = tc.tile_pool(name="wpool", bufs=1)
            wpool = wpool_ctx.__enter__()
            # Wo resident prefetch (needed only for the output projection)
            wo_sb = wpool.tile([128, 16 * D], BF16, tag="wosb")
            wov = wo_sb[:].rearrange("p (g o) -> p g o", o=D)
            for gg in range(4):
                nc.sync.dma_start(
                    wov[:, 4 * gg:4 * (gg + 1), :],
                    woT.ap()[512 * gg:512 * (gg + 1), :]
                    .rearrange("(g p) o -> p g o", p=128))

            a2a_in = {}
            a2a_out = {}
            oin = {}
            for h in range(2):
                a2a_in[h] = dram.tile([1024, 512], BF16, tag=f"a2ain{h}",
                                      name=f"a2a_in{h}")
                a2a_out[h] = dram.tile([1024, 512], BF16, tag=f"a2aout{h}",
                                       name=f"a2a_out{h}")
                oin[h] = wpool.tile([128, 8 * 512], BF16, tag=f"oin{h}",
                                    name=f"oin{h}")

            with tc.tile_pool(name="scps", bufs=2, space="PSUM") as scps, \
                 tc.tile_pool(name="pvps", bufs=2, space="PSUM") as pvps, \
                 tc.tile_pool(name="dnps", bufs=1, space="PSUM") as dnps, \
                 tc.tile_pool(name="bcps", bufs=1, space="PSUM") as bcps:
                pending = [None]

                def flush_tail():
                    if pending[0] is None:
                        return
                    pv, dn, h, b, j = pending[0]
                    pending[0] = None
                    rec = atop.tile([1, 512], F32, tag="rec",
                                     bufs=2, name="rec")
                    nc.vector.reciprocal_approx_fast(rec[:], dn[:])
                    bc = bcps.tile([128, 512], F32, tag="bc", name="bc")
                    nc.tensor.matmul(bc[:], ones_row[:], rec[:],
                                     start=True, stop=True)
                    bcb = atop.tile([128, 512], BF16, tag="bcbn",
                                     bufs=2, name="bcb")
                    nc.vector.tensor_copy(bcb[:], bc[:])
                    nrm = atop.tile([128, 512], BF16, tag="nrm",
                                     bufs=2, name="nrm")
                    nc.vector.tensor_mul(nrm[:], pv[:], bcb[:])
                    nc.sync.dma_start(
                        a2a_in[h][128 * (4 * b + j):
                                  128 * (4 * b + j + 1), :],
                        nrm[:])

                for h in range(2):
                    for b in range(B):
                        KT = roped[(b, 2)]
                        QT = roped[(b, h)]
                        vt = vT[b]
                        for j in range(NCB):
                            pv = pvps.tile([128, 512], F32, tag="pv",
                                           name="pv")
                            dn = dnps.tile([1, 512], F32, tag="dn",
                                           name="dn")
                            acc = atop.tile([128, 512], BF16, tag="acc",
                                             bufs=2, name="acc")
                            nprs = 2 * j + 2   # pairs of Sk blocks
                            pts = [None] * nprs
                            offp = [None] * nprs

                            def emit_qk(pr):
                                ps = scps.tile([128, 1024], F32, tag="sc",
                                               name="ps")
                                pt = atop.tile([128, 1024], BF16, tag="p",
                                                bufs=4, name="pt")
                                offs = []
                                for half in range(2):
                                    i = 2 * pr + half
                                    r = i - 4 * j
                                    off = 128 * max(r, 0)
                                    offs.append(off)
                                    base = 512 * half
                                    diag = (r >= 0)
                                    nc.tensor.matmul(
                                        ps[:, base + off:base + 512],
                                        KT[:, 128 * i:128 * (i + 1)],
                                        QT[:, 512 * j + off:512 * (j + 1)],
                                        start=True, stop=not diag)
                                    if diag:
                                        nc.tensor.matmul(
                                            ps[:, base + off:
                                               base + off + 128],
                                            mask_sb[:], id_sb[:],
                                            start=False, stop=True,
                                            skip_group_check=True)
                                if offs[1] > 0:
                                    nc.tensor.matmul(
                                        ps[:, 512:512 + offs[1]],
                                        negrow[:], onesb[:, 0:offs[1]],
                                        start=True, stop=True)
                                nc.scalar.activation(
                                    pt[:, offs[0]:1024],
                                    ps[:, offs[0]:1024], AF.Exp)
                                pts[pr] = pt
                                offp[pr] = offs
                                # denominator partials on DVE
                                if pr == 0:
                                    nc.vector.tensor_copy(
                                        acc[:], pt[:, 0:512])
                                else:
                                    nc.vector.tensor_add(
                                        acc[:, offs[0]:512],
                                        acc[:, offs[0]:512],
                                        pt[:, offs[0]:512])
                                nc.vector.tensor_add(
                                    acc[:, offs[1]:512],
                                    acc[:, offs[1]:512],
                                    pt[:, 512 + offs[1]:1024])

                            def emit_pv(pr):
                                pt = pts[pr]
                                offs = offp[pr]
                                for half in range(2):
                                    i = 2 * pr + half
                                    off = offs[half]
                                    first = (i == 0)
                                    last = (i == 4 * j + 3)
                                    base = 512 * half
                                    nc.tensor.matmul(
                                        pv[:, off:512],
                                        vt[:, 128 * i:128 * (i + 1)],
                                        pt[:, base + off:base + 512],
                                        start=first, stop=last,
                                        skip_group_check=True)

                            for pr in range(nprs):
                                emit_qk(pr)
                                if pr == min(1, nprs - 1):
                                    flush_tail()
                                if pr >= 1:
                                    emit_pv(pr - 1)
                            emit_pv(nprs - 1)
                            nc.tensor.matmul(dn[:], ones_col[:], acc[:],
                                             start=True, stop=True)
                            pending[0] = (pv, dn, h, b, j)
                    flush_tail()
                    nc.gpsimd.collective_compute(
                        "AllToAll", ALU.bypass,
                        replica_groups=[list(range(N_CORES))],
                        ins=[a2a_in[h].opt()], outs=[a2a_out[h].opt()],
                        cc_dim="Partition")
                    # gpsimd queue: keeps the sync queue free for the
                    # h=1 staging DMAs while the collective runs
                    nc.gpsimd.dma_start(
                        oin[h][:].rearrange("p (s t) -> p s t", t=512),
                        a2a_out[h][:].rearrange("(s p) t -> p s t", p=128))

            # ====================== out projection ====================
            # pass A: h=0 partial sums for all (n, mp) -> SBUF (runs during
            # the second all-to-all); pass B: h=1 partials + DVE combine.
            ovs = {h: oin[h][:].rearrange("p (s t) -> p s t", t=512)
                   for h in range(2)}
            with tc.tile_pool(name="opool", bufs=1) as opool, \
                 tc.tile_pool(name="ops", bufs=2, space="PSUM") as ops:
                ph0 = {}
                for n in range(4):
                    for mp in range(4):
                        pso = ops.tile([128, 512], F32, tag=f"oa{mp}",
                                       name=f"oa{mp}")
                        for s in range(8):
                            nc.tensor.matmul(
                                pso[:],
                                ovs[0][:, s, 128 * mp:128 * (mp + 1)],
                                wov[:, 2 * s, 512 * n:512 * (n + 1)],
                                start=(s == 0), stop=(s == 7))
                        pt0 = opool.tile([128, 512], F32, tag="ph0",
                                         bufs=16, name="pt0")
                        nc.scalar.copy(pt0[:], pso[:])
                        ph0[(n, mp)] = pt0
                for n in range(4):
                    for mp in range(4):
                        pso = ops.tile([128, 512], F32, tag=f"oa{mp}",
                                       name=f"ob{mp}")
                        for s in range(8):
                            nc.tensor.matmul(
                                pso[:],
                                ovs[1][:, s, 128 * mp:128 * (mp + 1)],
                                wov[:, 2 * s + 1, 512 * n:512 * (n + 1)],
                                start=(s == 0), stop=(s == 7))
                        os_t = opool.tile([128, 512], F32, tag="osb",
                                          bufs=4, name="os_t")
                        nc.vector.tensor_add(os_t[:], pso[:],
                                             ph0[(n, mp)][:])
                        nc.sync.dma_start(
                            out.ap()[128 * mp:128 * (mp + 1),
                                     512 * n:512 * (n + 1)],
                            os_t[:])
            wpool_ctx.__exit__(None, None, None)
            atop_ctx.__exit__(None, None, None)

    nc.compile()
    return nc


_NC_CACHE = None


def _get_nc():
    global _NC_CACHE
    if _NC_CACHE is None:
        _NC_CACHE = _build()
    return _NC_CACHE


def _host_prep(inputs):
    hs = np.asarray(inputs["hidden_states"], dtype=np.float32)
    Wq = np.asarray(inputs["Wq"], dtype=np.float32)
    Wk = np.asarray(inputs["Wk"], dtype=np.float32)
    Wv = np.asarray(inputs["Wv"], dtype=np.float32)
    Wo = np.asarray(inputs["Wo"], dtype=np.float32)
    cqw = np.asarray(inputs["canon_q_w"], dtype=np.float32)
    ckw = np.asarray(inputs["canon_k_w"], dtype=np.float32)
    cvw = np.asarray(inputs["canon_v_w"], dtype=np.float32)
    qnw = np.asarray(inputs["q_norm_w"], dtype=np.float32)
    knw = np.asarray(inputs["k_norm_w"], dtype=np.float32)

    bf = ml_dtypes.bfloat16
    hsT = np.ascontiguousarray(
        np.concatenate([hs[0].T, hs[1].T], axis=1)).astype(bf)
    WqT, WkT, WvT = Wq.T, Wk.T, Wv.T
    woT = np.ascontiguousarray(Wo.T).astype(bf)

    inv_freq = 1.0 / (10000.0 ** (np.arange(0, DH, 2, dtype=np.float64) / DH))
    freqs = np.arange(S, dtype=np.float64)[:, None] * inv_freq
    emb = np.concatenate([freqs, freqs], axis=-1)
    cosT, sinT = np.cos(emb).T, np.sin(emb).T

    def make_rope(normw, scale):
        A = cosT * normw[:, None] * scale
        wswap = normw[(np.arange(DH) + 64) % DH]
        sign = np.where(np.arange(DH) < 64, -1.0, 1.0)
        Bc = sinT * wswap[:, None] * sign[:, None] * scale
        return (np.ascontiguousarray(A).astype(bf),
                np.ascontiguousarray(Bc).astype(bf))

    Aq, Bq = make_rope(qnw, SCALE)
    Ak, Bk = make_rope(knw, 1.0)

    p = np.arange(128)[:, None]
    f = np.arange(128)[None, :]
    maskd = np.where(p <= f, 0.0, NEG).astype(np.float32)
    maskTb = np.ascontiguousarray(maskd.T).astype(bf)
    idb = np.eye(128, dtype=np.float32).astype(bf)

    in_maps = []
    for r in range(N_CORES):
        wTc = np.ascontiguousarray(np.concatenate(
            [WqT[:, 256 * r:256 * r + 256],
             WkT[:, 128 * r:128 * r + 128],
             WvT[:, 128 * r:128 * r + 128]], axis=1)).astype(bf)
        cwc = np.ascontiguousarray(np.concatenate(
            [cqw[256 * r:256 * r + 256],
             ckw[128 * r:128 * r + 128],
             cvw[128 * r:128 * r + 128]], axis=0)).astype(np.float32)
        in_maps.append({
            "hsT": hsT, "wT": wTc, "woT": woT, "cw": cwc,
            "ropeAq": Aq, "ropeBq": Bq, "ropeAk": Ak, "ropeBk": Bk,
            "maskTb": maskTb, "idb": idb,
        })
    return in_maps


def kernel(**inputs):
    nc = _get_nc()
    in_maps = _host_prep(inputs)
    res = run_bass_kernel_spmd(nc, in_maps, core_ids=list(range(N_CORES)))
    full = np.empty((B, S, D), np.float32)
    for r in range(N_CORES):
        full[r // 4, 512 * (r % 4):512 * (r % 4 + 1), :] = res.results[r]["out"]
    return full


# revision 46
# speedup vs baseline: 1.0455x; 1.0455x over previous
"""Trainium2 Bass kernel for CanonCausalMultiheadAttn.

Sharding: tensor-parallel over heads across 8 cores (2 q-heads + 1 kv-head
per core), both batches replicated. Two head-split AllToAlls exchange
attention outputs so each core owns one (batch, seq-slice) of the final
output projection; the first overlaps the second half of attention and the
second overlaps the h=0 partial pass of the output projection.

Per-core pipeline (shapes hardcoded for B=2, S=2048, D=2048):
  QKV proj (bf16 matmul, weights SBUF-resident) -> canon conv via halo'd
  raw buffer (DVE, bf16) -> qk rmsnorm rstd via PE column-sum matmuls and
  fast-approx reciprocal -> RoPE (DVE bf16; norm-weight & 1/sqrt(dh)
  folded into host tables; q AND k rstd folded in via K=1 broadcast
  matmuls whose emission is software-pipelined into later PE-dense
  phases) -> causal attention with scores in [Sk, Sq] layout, two
  Sk-blocks paired per [128,1024] PSUM tile so one exp covers both; the
  causal mask and the fully-masked wedge are added on the PE itself
  (maskT.T @ I accumulation) so exp depends only on the PE; PV computed
  transposed (V stationary) directly in [dh, Sq]; softmax denominator via
  ones-column matmuls; normalize tail software-pipelined into the next
  block -> AllToAll x2 (head-split) -> output projection in two passes
  (h=0 partials to SBUF during the second AllToAll, then h=1 + combine).
"""
import sys

sys.path.insert(0, '/opt/trn_rl_repo')

import numpy as np
import ml_dtypes

import concourse.bass as bass
import concourse.mybir as mybir
import concourse.tile as tile
from concourse import bacc
from concourse.bass_utils import run_bass_kernel_spmd

F32 = mybir.dt.float32
F32R = mybir.dt.float32r
BF16 = mybir.dt.bfloat16
AF = mybir.ActivationFunctionType
ALU = mybir.AluOpType

B, S, D = 2, 2048, 2048
NH, NKV, DH = 16, 8, 128
K_CONV = 4
EPS = 1e-6
SCALE = 1.0 / float(np.sqrt(DH))
NEG = -1e9
N_CORES = 8
NCB = S // 512          # 512-token chunks per batch
N_SKB = S // 128        # Sk blocks per batch


def _build():
    nc = bacc.Bacc("TRN2", target_bir_lowering=False, debug=False,
                   num_devices=N_CORES)

    hsT = nc.dram_tensor("hsT", [D, B * S], BF16, kind="ExternalInput")
    wT = nc.dram_tensor("wT", [D, 512], BF16, kind="ExternalInput")
    woT = nc.dram_tensor("woT", [D, D], BF16, kind="ExternalInput")
    cw = nc.dram_tensor("cw", [512, K_CONV], F32, kind="ExternalInput")
    ropeAq = nc.dram_tensor("ropeAq", [DH, S], BF16, kind="ExternalInput")
    ropeBq = nc.dram_tensor("ropeBq", [DH, S], BF16, kind="ExternalInput")
    ropeAk = nc.dram_tensor("ropeAk", [DH, S], BF16, kind="ExternalInput")
    ropeBk = nc.dram_tensor("ropeBk", [DH, S], BF16, kind="ExternalInput")
    maskTb = nc.dram_tensor("maskTb", [128, 128], BF16, kind="ExternalInput")
    idb = nc.dram_tensor("idb", [128, 128], BF16, kind="ExternalInput")
    out = nc.dram_tensor("out", [512, D], F32, kind="ExternalOutput")

    with tile.TileContext(nc) as tc:
        with tc.tile_pool(name="const", bufs=1) as cpool, \
             tc.tile_pool(name="persist", bufs=1) as pers, \
             tc.tile_pool(name="dram", bufs=1, space="DRAM") as dram:

            # QKV weights resident in SBUF: [128, 16 k-blocks x 512]
            wT_sb = cpool.tile([128, 16 * 512], BF16, tag="wTsb")
            wv = wT_sb[:].rearrange("p (k c) -> p k c", c=512)
            for kk in range(4):
                nc.sync.dma_start(
                    wv[:, 4 * kk:4 * (kk + 1), :],
                    wT.ap()[512 * kk:512 * (kk + 1), :]
                    .rearrange("(k p) c -> p k c", p=128))

            # ---- constants (rope tables last: not needed until late) ----
            mask_sb = cpool.tile([128, 128], BF16, tag="mask")
            nc.sync.dma_start(mask_sb[:], maskTb.ap())
            id_sb = cpool.tile([128, 128], BF16, tag="idsb")
            nc.sync.dma_start(id_sb[:], idb.ap())
            cw_sb = []
            for mt in range(4):
                t = cpool.tile([128, K_CONV], F32, tag=f"cw{mt}",
                               name=f"cw{mt}")
                nc.sync.dma_start(t[:], cw.ap()[128 * mt:128 * mt + 128, :])
                cw_sb.append(t)
            ones_col = cpool.tile([128, 1], BF16, tag="oc")
            nc.vector.memset(ones_col[:], 1.0)
            eps_sb = cpool.tile([1, 1], F32, tag="eps")
            nc.vector.memset(eps_sb[:], EPS)
            ones_row = cpool.tile([1, 128], F32, tag="or")
            nc.vector.memset(ones_row[:], 1.0)
            # NEG broadcast: negrow.T @ onesb[:, :w] == NEG everywhere
            negrow = cpool.tile([128, 128], BF16, tag="negrow")
            nc.vector.memset(negrow[:], 0.0)
            nc.vector.memset(negrow[0:1, :], NEG)
            onesb = cpool.tile([128, 512], BF16, tag="onesb")
            nc.vector.memset(onesb[:], 1.0)
            s0_sb = []
            for mt in range(4):
                t = cpool.tile([128, 1], F32, tag=f"s0{mt}", name=f"s0{mt}")
                nc.vector.tensor_scalar_add(t[:], cw_sb[mt][:, 0:1], 1.0)
                s0_sb.append(t)
            ropes = {}
            for nm in ("Aq", "Bq", "Ak", "Bk"):
                ropes[nm] = cpool.tile([DH, S], BF16, tag=f"rope{nm}",
                                       name=f"rope{nm}")

            # persistent per-(b,mt) tiles
            roped = {}   # (b, mt) -> [128, S] bf16 (rstd folded in)
            vT = {}      # b -> [128, N_SKB*128] bf16 (V transposed blocks)

            for b in range(B):
                vT[b] = pers.tile([128, N_SKB * 128], BF16, tag=f"vT{b}",
                                  name=f"vT{b}")
                for mt in range(3):
                    roped[(b, mt)] = pers.tile(
                        [128, S], BF16, tag=f"roped{b}{mt}",
                        name=f"roped{b}{mt}")

            # ============ phase Q: QKV + canon + norm + rope ============
            # attention working tiles live at top level so they never
            # land on recycled phase-Q scratch space (avoids end-of-phase
            # write-after-read stalls)
            atop_ctx = tc.tile_pool(name="atop", bufs=1)
            atop = atop_ctx.__enter__()

            with tc.tile_pool(name="qps", bufs=1, space="PSUM") as qps, \
                 tc.tile_pool(name="spp", bufs=2, space="PSUM") as spp, \
                 tc.tile_pool(name="bps", bufs=2, space="PSUM") as bps, \
                 tc.tile_pool(name="bwork", bufs=1) as bw:
                cn = {}
                raw_h = {}
                for mt in range(4):
                    cn[mt] = bw.tile([128, S], BF16, tag=f"cn{mt}",
                                     name=f"cn{mt}")
                    raw_h[mt] = bw.tile([128, 516], BF16,
                                        tag=f"rawh{mt}",
                                        name=f"raw_h{mt}")
                psums = [qps.tile([128, 512], F32, tag=f"qk{mt}",
                                  name=f"qk{mt}") for mt in range(4)]
                for b in range(B):
                    if True:
                        for mt in range(4):
                            nc.vector.memset(raw_h[mt][:, 0:4], 0.0)

                        def emit_chunk_mms(n):
                            hs_sb = bw.tile([128, 16 * 512], BF16,
                                            tag="hschunk", bufs=2,
                                            name="hs_sb")
                            hsv = hs_sb[:].rearrange("p (k s) -> p k s",
                                                     s=512)
                            for kh in range(2):
                                nc.sync.dma_start(
                                    hsv[:, 8 * kh:8 * (kh + 1), :],
                                    hsT.ap()[1024 * kh:1024 * (kh + 1),
                                             b * S + 512 * n:
                                             b * S + 512 * (n + 1)]
                                    .rearrange("(k p) s -> p k s", p=128))
                            hvv = hs_sb[:].rearrange("p (k s) -> p k s",
                                                     s=512)
                            for k in range(16):
                                for mt in range(4):
                                    nc.tensor.matmul(
                                        psums[mt][:],
                                        wv[:, k, 128 * mt:128 * (mt + 1)],
                                        hvv[:, k, :],
                                        start=(k == 0), stop=(k == 15))
                            if b == 0 and n == 1:
                                for nm, t in (("Aq", ropeAq), ("Bq", ropeBq),
                                              ("Ak", ropeAk), ("Bk", ropeBk)):
                                    nc.sync.dma_start(ropes[nm][:], t.ap())

                        def emit_raw(n):
                            for mt in range(4):
                                rh = raw_h[mt]
                                if n > 0:
                                    nc.vector.tensor_copy(
                                        rh[:, 1:4], rh[:, 513:516])
                                nc.vector.tensor_copy(rh[:, 4:516],
                                                      psums[mt][:])

                        def emit_canon(n):
                            lo = 512 * n
                            for mt in range(4):
                                rh = raw_h[mt]
                                c = cn[mt]
                                nc.vector.tensor_scalar_mul(
                                    c[:, lo:lo + 512], rh[:, 4:516],
                                    s0_sb[mt][:])
                                for k in range(1, K_CONV):
                                    nc.vector.scalar_tensor_tensor(
                                        c[:, lo:lo + 512],
                                        rh[:, 4 - k:516 - k],
                                        cw_sb[mt][:, k:k + 1],
                                        c[:, lo:lo + 512],
                                        ALU.mult, ALU.add)
                            # squares for rmsnorm (q0, q1, k)
                            for mt in range(3):
                                sq = bw.tile([128, 512], BF16, tag="sqr",
                                             bufs=3, name="sq")
                                nc.vector.tensor_mul(
                                    sq[:], cn[mt][:, lo:lo + 512],
                                    cn[mt][:, lo:lo + 512])
                                sqs[(n, mt)] = sq

                        def emit_norm(n):
                            for mt in range(3):
                                sp = spp.tile([1, 512], F32, tag="ssq")
                                nc.tensor.matmul(sp[:], ones_col[:],
                                                 sqs.pop((n, mt))[:],
                                                 start=True, stop=True)
                                srt = bw.tile([1, 512], F32, tag="srt",
                                              bufs=2, name="srt")
                                nc.scalar.activation(srt[:], sp[:], AF.Sqrt,
                                                     bias=eps_sb[:],
                                                     scale=1.0 / DH)
                                rn = bw.tile([1, 512], F32, tag=f"rn{mt}",
                                             bufs=3, name=f"rn{mt}")
                                nc.vector.reciprocal_approx_fast(
                                    rn[:], srt[:])
                                rns[(n, mt)] = rn
                            # V transpose blocks for this chunk
                            for t in range(4):
                                i = 4 * n + t
                                nc.sync.dma_start_transpose(
                                    vT[b][:, 128 * i:128 * (i + 1)],
                                    cn[3][:, 128 * i:128 * (i + 1)])

                        def emit_bc(n):
                            # rstd broadcast for chunk n (PE + ACT only;
                            # emitted inside the PE-dense chunk stream)
                            for mt in range(3):
                                bp = bps.tile([128, 512], F32, tag="bcp")
                                nc.tensor.matmul(
                                    bp[:], ones_row[:],
                                    rns.pop((n, mt))[:],
                                    start=True, stop=True)
                                bcb = bw.tile([128, 512], BF16, tag="bcs",
                                              bufs=4, name="bcs")
                                nc.scalar.copy(bcb[:], bp[:])
                                bcb_store[(mt, n)] = bcb

                        def emit_rope1(n):
                            # chunk-wise rope + in-place rstd scale so
                            # roped is complete when the pipeline drains
                            lo = 512 * n
                            for mt in (2, 0, 1):
                                is_q = mt < 2
                                x = cn[mt]
                                A_ = ropes["Aq"] if is_q else ropes["Ak"]
                                B_ = ropes["Bq"] if is_q else ropes["Bk"]
                                sh = bw.tile([128, 512], BF16, tag="shift",
                                             bufs=3, name="sh")
                                nc.gpsimd.dma_start(sh[0:64, :],
                                                    x[64:128, lo:lo + 512])
                                nc.gpsimd.dma_start(sh[64:128, :],
                                                    x[0:64, lo:lo + 512])
                                nc.vector.tensor_mul(sh[:], sh[:],
                                                     B_[:, lo:lo + 512])
                                ro = roped[(b, mt)]
                                nc.vector.tensor_mul(ro[:, lo:lo + 512],
                                                     x[:, lo:lo + 512],
                                                     A_[:, lo:lo + 512])
                                nc.vector.tensor_add(ro[:, lo:lo + 512],
                                                     ro[:, lo:lo + 512],
                                                     sh[:])
                                nc.vector.tensor_mul(
                                    ro[:, lo:lo + 512],
                                    ro[:, lo:lo + 512],
                                    bcb_store.pop((mt, n))[:])

                        sqs = {}
                        rns = {}
                        bcb_store = {}
                        for n in range(NCB + 2):
                            if n < NCB:
                                emit_chunk_mms(n)
                            if 1 <= n <= NCB:
                                emit_norm(n - 1)
                            if n >= 2:
                                emit_bc(n - 2)
                            if n < NCB:
                                emit_raw(n)
                                emit_canon(n)
                            if n >= 2:
                                emit_rope1(n - 2)

            # ============ attention + head-split all-to-all ============
            wpool_ctx = tc.tile_pool(name="wpool", bufs=1)
            wpool = wpool_ctx.__enter__()
            # Wo resident prefetch (needed only for the output projection)
            wo_sb = wpool.tile([128, 16 * D], BF16, tag="wosb")
            wov = wo_sb[:].rearrange("p (g o) -> p g o", o=D)
            for gg in range(4):
                nc.sync.dma_start(
                    wov[:, 4 * gg:4 * (gg + 1), :],
                    woT.ap()[512 * gg:512 * (gg + 1), :]
                    .rearrange("(g p) o -> p g o", p=128))

            a2a_in = {}
            a2a_out = {}
            oin = {}
            for h in range(2):
                a2a_in[h] = dram.tile([1024, 512], BF16, tag=f"a2ain{h}",
                                      name=f"a2a_in{h}")
                a2a_out[h] = dram.tile([1024, 512], BF16, tag=f"a2aout{h}",
                                       name=f"a2a_out{h}")
                oin[h] = wpool.tile([128, 8 * 512], BF16, tag=f"oin{h}",
                                    name=f"oin{h}")

            with tc.tile_pool(name="scps", bufs=2, space="PSUM") as scps, \
                 tc.tile_pool(name="pvps", bufs=2, space="PSUM") as pvps, \
                 tc.tile_pool(name="dnps", bufs=1, space="PSUM") as dnps, \
                 tc.tile_pool(name="bcps", bufs=1, space="PSUM") as bcps:
                pending = [None]

                def flush_tail():
                    if pending[0] is None:
                        return
                    pv, dn, h, b, j = pending[0]
                    pending[0] = None
                    rec = atop.tile([1, 512], F32, tag="rec",
                                     bufs=2, name="rec")
                    nc.vector.reciprocal_approx_fast(rec[:], dn[:])
                    bc = bcps.tile([128, 512], F32, tag="bc", name="bc")
                    nc.tensor.matmul(bc[:], ones_row[:], rec[:],
                                     start=True, stop=True)
                    bcb = atop.tile([128, 512], BF16, tag="bcbn",
                                     bufs=2, name="bcb")
                    nc.vector.tensor_copy(bcb[:], bc[:])
                    nrm = atop.tile([128, 512], BF16, tag="nrm",
                                     bufs=2, name="nrm")
                    nc.vector.tensor_mul(nrm[:], pv[:], bcb[:])
                    nc.sync.dma_start(
                        a2a_in[h][128 * (4 * b + j):
                                  128 * (4 * b + j + 1), :],
                        nrm[:])

                for h in range(2):
                    for b in range(B):
                        KT = roped[(b, 2)]
                        QT = roped[(b, h)]
                        vt = vT[b]
                        for j in range(NCB):
                            pv = pvps.tile([128, 512], F32, tag="pv",
                                           name="pv")
                            dn = dnps.tile([1, 512], F32, tag="dn",
                                           name="dn")
                            acc = atop.tile([128, 512], BF16, tag="acc",
                                             bufs=2, name="acc")
                            nprs = 2 * j + 2   # pairs of Sk blocks
                            pts = [None] * nprs
                            offp = [None] * nprs

                            def emit_qk(pr):
                                ps = scps.tile([128, 1024], F32, tag="sc",
                                               name="ps")
                                pt = atop.tile([128, 1024], BF16, tag="p",
                                                bufs=4, name="pt")
                                offs = []
                                for half in range(2):
                                    i = 2 * pr + half
                                    r = i - 4 * j
                                    off = 128 * max(r, 0)
                                    offs.append(off)
                                    base = 512 * half
                                    diag = (r >= 0)
                                    nc.tensor.matmul(
                                        ps[:, base + off:base + 512],
                                        KT[:, 128 * i:128 * (i + 1)],
                                        QT[:, 512 * j + off:512 * (j + 1)],
                                        start=True, stop=not diag)
                                    if diag:
                                        nc.tensor.matmul(
                                            ps[:, base + off:
                                               base + off + 128],
                                            mask_sb[:], id_sb[:],
                                            start=False, stop=True,
                                            skip_group_check=True)
                                if offs[1] > 0:
                                    nc.tensor.matmul(
                                        ps[:, 512:512 + offs[1]],
                                        negrow[:], onesb[:, 0:offs[1]],
                                        start=True, stop=True)
                                nc.scalar.activation(
                                    pt[:, offs[0]:1024],
                                    ps[:, offs[0]:1024], AF.Exp)
                                pts[pr] = pt
                                offp[pr] = offs
                                # denominator partials on DVE
                                if pr == 0:
                                    nc.vector.tensor_copy(
                                        acc[:], pt[:, 0:512])
                                else:
                                    nc.vector.tensor_add(
                                        acc[:, offs[0]:512],
                                        acc[:, offs[0]:512],
                                        pt[:, offs[0]:512])
                                nc.vector.tensor_add(
                                    acc[:, offs[1]:512],
                                    acc[:, offs[1]:512],
                                    pt[:, 512 + offs[1]:1024])

                            def emit_pv(pr):
                                pt = pts[pr]
                                offs = offp[pr]
                                for half in range(2):
                                    i = 2 * pr + half
                                    off = offs[half]
                                    first = (i == 0)
                                    last = (i == 4 * j + 3)
                                    base = 512 * half
                                    nc.tensor.matmul(
                                        pv[:, off:512],
                                        vt[:, 128 * i:128 * (i + 1)],
                                        pt[:, base + off:base + 512],
                                        start=first, stop=last,
                                        skip_group_check=True)

                            for pr in range(nprs):
                                emit_qk(pr)
                                if pr == min(1, nprs - 1):
                                    flush_tail()
                                if pr >= 1:
                                    emit_pv(pr - 1)
                            emit_pv(nprs - 1)
                            nc.tensor.matmul(dn[:], ones_col[:], acc[:],
                                             start=True, stop=True)
                            pending[0] = (pv, dn, h, b, j)
                    flush_tail()
                    nc.gpsimd.collective_compute(
                        "AllToAll", ALU.bypass,
                        replica_groups=[list(range(N_CORES))],
                        ins=[a2a_in[h].opt()], outs=[a2a_out[h].opt()],
                        cc_dim="Partition")
                    # gpsimd queue: keeps the sync queue free for the
                    # h=1 staging DMAs while the collective runs
                    nc.gpsimd.dma_start(
                        oin[h][:].rearrange("p (s t) -> p s t", t=512),
                        a2a_out[h][:].rearrange("(s p) t -> p s t", p=128))

            # ====================== out projection ====================
            # pass A: h=0 partial sums for all (n, mp) -> SBUF (runs during
            # the second all-to-all); pass B: h=1 partials + DVE combine.
            ovs = {h: oin[h][:].rearrange("p (s t) -> p s t", t=512)
                   for h in range(2)}
            with tc.tile_pool(name="opool", bufs=1) as opool, \
                 tc.tile_pool(name="ops", bufs=2, space="PSUM") as ops:
                ph0 = {}
                for n in range(4):
                    for mp in range(4):
                        pso = ops.tile([128, 512], F32, tag=f"oa{mp}",
                                       name=f"oa{mp}")
                        for s in range(8):
                            nc.tensor.matmul(
                                pso[:],
                                ovs[0][:, s, 128 * mp:128 * (mp + 1)],
                                wov[:, 2 * s, 512 * n:512 * (n + 1)],
                                start=(s == 0), stop=(s == 7))
                        pt0 = opool.tile([128, 512], F32, tag="ph0",
                                         bufs=16, name="pt0")
                        nc.scalar.copy(pt0[:], pso[:])
                        ph0[(n, mp)] = pt0
                for n in range(4):
                    for mp in range(4):
                        pso = ops.tile([128, 512], F32, tag=f"oa{mp}",
                                       name=f"ob{mp}")
                        for s in range(8):
                            nc.tensor.matmul(
                                pso[:],
                                ovs[1][:, s, 128 * mp:128 * (mp + 1)],
                                wov[:, 2 * s + 1, 512 * n:512 * (n + 1)],
                                start=(s == 0), stop=(s == 7))
                        os_t = opool.tile([128, 512], F32, tag="osb",
                                          bufs=4, name="os_t")
                        nc.vector.tensor_add(os_t[:], pso[:],
                                             ph0[(n, mp)][:])
                        nc.sync.dma_start(
                            out.ap()[128 * mp:128 * (mp + 1),
                                     512 * n:512 * (n + 1)],
                            os_t[:])
            wpool_ctx.__exit__(None, None, None)
            atop_ctx.__exit__(None, None, None)

    nc.compile()
    return nc


_NC_CACHE = None


def _get_nc():
    global _NC_CACHE
    if _NC_CACHE is None:
        _NC_CACHE = _build()
    return _NC_CACHE


def _host_prep(inputs):
    hs = np.asarray(inputs["hidden_states"], dtype=np.float32)
    Wq = np.asarray(inputs["Wq"], dtype=np.float32)
    Wk = np.asarray(inputs["Wk"], dtype=np.float32)
    Wv = np.asarray(inputs["Wv"], dtype=np.float32)
    Wo = np.asarray(inputs["Wo"], dtype=np.float32)
    cqw = np.asarray(inputs["canon_q_w"], dtype=np.float32)
    ckw = np.asarray(inputs["canon_k_w"], dtype=np.float32)
    cvw = np.asarray(inputs["canon_v_w"], dtype=np.float32)
    qnw = np.asarray(inputs["q_norm_w"], dtype=np.float32)
    knw = np.asarray(inputs["k_norm_w"], dtype=np.float32)

    bf = ml_dtypes.bfloat16
    hsT = np.ascontiguousarray(
        np.concatenate([hs[0].T, hs[1].T], axis=1)).astype(bf)
    WqT, WkT, WvT = Wq.T, Wk.T, Wv.T
    woT = np.ascontiguousarray(Wo.T).astype(bf)

    inv_freq = 1.0 / (10000.0 ** (np.arange(0, DH, 2, dtype=np.float64) / DH))
    freqs = np.arange(S, dtype=np.float64)[:, None] * inv_freq
    emb = np.concatenate([freqs, freqs], axis=-1)
    cosT, sinT = np.cos(emb).T, np.sin(emb).T

    def make_rope(normw, scale):
        A = cosT * normw[:, None] * scale
        wswap = normw[(np.arange(DH) + 64) % DH]
        sign = np.where(np.arange(DH) < 64, -1.0, 1.0)
        Bc = sinT * wswap[:, None] * sign[:, None] * scale
        return (np.ascontiguousarray(A).astype(bf),
                np.ascontiguousarray(Bc).astype(bf))

    Aq, Bq = make_rope(qnw, SCALE)
    Ak, Bk = make_rope(knw, 1.0)

    p = np.arange(128)[:, None]
    f = np.arange(128)[None, :]
    maskd = np.where(p <= f, 0.0, NEG).astype(np.float32)
    maskTb = np.ascontiguousarray(maskd.T).astype(bf)
    idb = np.eye(128, dtype=np.float32).astype(bf)

    in_maps = []
    for r in range(N_CORES):
        wTc = np.ascontiguousarray(np.concatenate(
            [WqT[:, 256 * r:256 * r + 256],
             WkT[:, 128 * r:128 * r + 128],
             WvT[:, 128 * r:128 * r + 128]], axis=1)).astype(bf)
        cwc = np.ascontiguousarray(np.concatenate(
            [cqw[256 * r:256 * r + 256],
             ckw[128 * r:128 * r + 128],
             cvw[128 * r:128 * r + 128]], axis=0)).astype(np.float32)
        in_maps.append({
            "hsT": hsT, "wT": wTc, "woT": woT, "cw": cwc,
            "ropeAq": Aq, "ropeBq": Bq, "ropeAk": Ak, "ropeBk": Bk,
            "maskTb": maskTb, "idb": idb,
        })
    return in_maps


def kernel(**inputs):
    nc = _get_nc()
    in_maps = _host_prep(inputs)
    res = run_bass_kernel_spmd(nc, in_maps, core_ids=list(range(N_CORES)))
    full = np.empty((B, S, D), np.float32)
    for r in range(N_CORES):
        full[r // 4, 512 * (r % 4):512 * (r % 4 + 1), :] = res.results[r]["out"]
    return full


# revision 50
# speedup vs baseline: 1.0743x; 1.0275x over previous
"""Trainium2 Bass kernel for CanonCausalMultiheadAttn.

Sharding: tensor-parallel over heads across 8 cores (2 q-heads + 1 kv-head
per core), both batches replicated. Two head-split AllToAlls exchange
attention outputs so each core owns one (batch, seq-slice) of the final
output projection; the first overlaps the second half of attention and the
second overlaps the h=0 partial pass of the output projection.

Per-core pipeline (shapes hardcoded for B=2, S=2048, D=2048):
  QKV proj (bf16 matmul, weights SBUF-resident) -> canon conv via halo'd
  raw buffer (DVE, bf16) -> qk rmsnorm rstd via PE column-sum matmuls and
  fast-approx reciprocal -> RoPE (DVE bf16; norm-weight & 1/sqrt(dh)
  folded into host tables; q AND k rstd folded in via K=1 broadcast
  matmuls whose emission is software-pipelined into later PE-dense
  phases) -> causal attention with scores in [Sk, Sq] layout, two
  Sk-blocks paired per [128,1024] PSUM tile so one exp covers both; the
  causal mask and the fully-masked wedge are added on the PE itself
  (maskT.T @ I accumulation) so exp depends only on the PE; PV computed
  transposed (V stationary) directly in [dh, Sq]; softmax denominator via
  ones-column matmuls; normalize tail software-pipelined into the next
  block -> AllToAll x2 (head-split) -> output projection in two passes
  (h=0 partials to SBUF during the second AllToAll, then h=1 + combine).
"""
import sys

sys.path.insert(0, '/opt/trn_rl_repo')

import numpy as np
import ml_dtypes

import concourse.bass as bass
import concourse.mybir as mybir
import concourse.tile as tile
from concourse import bacc
from concourse.bass_utils import run_bass_kernel_spmd

F32 = mybir.dt.float32
F32R = mybir.dt.float32r
BF16 = mybir.dt.bfloat16
AF = mybir.ActivationFunctionType
ALU = mybir.AluOpType

B, S, D = 2, 2048, 2048
NH, NKV, DH = 16, 8, 128
K_CONV = 4
EPS = 1e-6
SCALE = 1.0 / float(np.sqrt(DH))
NEG = -1e9
N_CORES = 8
NCB = S // 512          # 512-token chunks per batch
N_SKB = S // 128        # Sk blocks per batch


def _build():
    nc = bacc.Bacc("TRN2", target_bir_lowering=False, debug=False,
                   num_devices=N_CORES)

    hsT = nc.dram_tensor("hsT", [D, B * S], BF16, kind="ExternalInput")
    wT = nc.dram_tensor("wT", [D, 512], BF16, kind="ExternalInput")
    woT = nc.dram_tensor("woT", [D, D], BF16, kind="ExternalInput")
    cw = nc.dram_tensor("cw", [512, K_CONV], F32, kind="ExternalInput")
    ropeAq = nc.dram_tensor("ropeAq", [DH, S], BF16, kind="ExternalInput")
    ropeBq = nc.dram_tensor("ropeBq", [DH, S], BF16, kind="ExternalInput")
    ropeAk = nc.dram_tensor("ropeAk", [DH, S], BF16, kind="ExternalInput")
    ropeBk = nc.dram_tensor("ropeBk", [DH, S], BF16, kind="ExternalInput")
    maskTb = nc.dram_tensor("maskTb", [128, 128], BF16, kind="ExternalInput")
    idb = nc.dram_tensor("idb", [128, 128], BF16, kind="ExternalInput")
    out = nc.dram_tensor("out", [512, D], F32, kind="ExternalOutput")

    with tile.TileContext(nc) as tc:
        with tc.tile_pool(name="const", bufs=1) as cpool, \
             tc.tile_pool(name="persist", bufs=1) as pers, \
             tc.tile_pool(name="dram", bufs=1, space="DRAM") as dram:

            # canon weights first (tiny, needed by the first chunk's canon)
            cw_sb = []
            for mt in range(4):
                t = cpool.tile([128, K_CONV], F32, tag=f"cw{mt}",
                               name=f"cw{mt}")
                nc.sync.dma_start(t[:], cw.ap()[128 * mt:128 * mt + 128, :])
                cw_sb.append(t)

            # QKV weights resident in SBUF: [128, 16 k-blocks x 512];
            # per-k DMAs so the first matmul waits only on block 0
            wT_sb = cpool.tile([128, 16 * 512], BF16, tag="wTsb")
            wv = wT_sb[:].rearrange("p (k c) -> p k c", c=512)
            for kk in range(16):
                nc.sync.dma_start(
                    wv[:, kk:kk + 1, :],
                    wT.ap()[128 * kk:128 * (kk + 1), :]
                    .rearrange("(k p) c -> p k c", p=128))

            # ---- constants (rope tables last: not needed until late) ----
            mask_sb = cpool.tile([128, 128], BF16, tag="mask")
            id_sb = cpool.tile([128, 128], BF16, tag="idsb")
            ones_col = cpool.tile([128, 1], BF16, tag="oc")
            nc.vector.memset(ones_col[:], 1.0)
            eps_sb = cpool.tile([1, 1], F32, tag="eps")
            nc.vector.memset(eps_sb[:], EPS)
            ones_row = cpool.tile([1, 128], F32, tag="or")
            nc.vector.memset(ones_row[:], 1.0)
            # NEG broadcast: negrow.T @ onesb[:, :w] == NEG everywhere
            negrow = cpool.tile([128, 128], BF16, tag="negrow")
            nc.vector.memset(negrow[:], 0.0)
            nc.vector.memset(negrow[0:1, :], NEG)
            onesb = cpool.tile([128, 512], BF16, tag="onesb")
            nc.vector.memset(onesb[:], 1.0)
            s0_sb = []
            for mt in range(4):
                t = cpool.tile([128, 1], F32, tag=f"s0{mt}", name=f"s0{mt}")
                nc.vector.tensor_scalar_add(t[:], cw_sb[mt][:, 0:1], 1.0)
                s0_sb.append(t)
            ropes = {}
            for nm in ("Aq", "Bq", "Ak", "Bk"):
                ropes[nm] = cpool.tile([DH, S], BF16, tag=f"rope{nm}",
                                       name=f"rope{nm}")

            # persistent per-(b,mt) tiles
            roped = {}   # (b, mt) -> [128, S] bf16 (rstd folded in)
            vT = {}      # b -> [128, N_SKB*128] bf16 (V transposed blocks)

            for b in range(B):
                vT[b] = pers.tile([128, N_SKB * 128], BF16, tag=f"vT{b}",
                                  name=f"vT{b}")
                for mt in range(3):
                    roped[(b, mt)] = pers.tile(
                        [128, S], BF16, tag=f"roped{b}{mt}",
                        name=f"roped{b}{mt}")

            # ============ phase Q: QKV + canon + norm + rope ============
            # attention working tiles live at top level so they never
            # land on recycled phase-Q scratch space (avoids end-of-phase
            # write-after-read stalls)
            atop_ctx = tc.tile_pool(name="atop", bufs=1)
            atop = atop_ctx.__enter__()

            with tc.tile_pool(name="qps", bufs=1, space="PSUM") as qps, \
                 tc.tile_pool(name="spp", bufs=2, space="PSUM") as spp, \
                 tc.tile_pool(name="bps", bufs=2, space="PSUM") as bps, \
                 tc.tile_pool(name="bwork", bufs=1) as bw:
                cn = {}
                raw_h = {}
                for mt in range(4):
                    cn[mt] = bw.tile([128, S], BF16, tag=f"cn{mt}",
                                     name=f"cn{mt}")
                    raw_h[mt] = bw.tile([128, 516], BF16,
                                        tag=f"rawh{mt}",
                                        name=f"raw_h{mt}")
                psums = [qps.tile([128, 512], F32, tag=f"qk{mt}",
                                  name=f"qk{mt}") for mt in range(4)]
                for b in range(B):
                    if True:
                        for mt in range(4):
                            nc.vector.memset(raw_h[mt][:, 0:4], 0.0)

                        def emit_chunk_mms(n):
                            hs_sb = bw.tile([128, 16 * 512], BF16,
                                            tag="hschunk", bufs=2,
                                            name="hs_sb")
                            hsv = hs_sb[:].rearrange("p (k s) -> p k s",
                                                     s=512)
                            nsp = 4 if (b == 0 and n == 0) else 2
                            kb = 16 // nsp
                            for kh in range(nsp):
                                nc.sync.dma_start(
                                    hsv[:, kb * kh:kb * (kh + 1), :],
                                    hsT.ap()[128 * kb * kh:
                                             128 * kb * (kh + 1),
                                             b * S + 512 * n:
                                             b * S + 512 * (n + 1)]
                                    .rearrange("(k p) s -> p k s", p=128))
                            hvv = hs_sb[:].rearrange("p (k s) -> p k s",
                                                     s=512)
                            for k in range(16):
                                for mt in range(4):
                                    nc.tensor.matmul(
                                        psums[mt][:],
                                        wv[:, k, 128 * mt:128 * (mt + 1)],
                                        hvv[:, k, :],
                                        start=(k == 0), stop=(k == 15))
                            if b == 0 and n == 1:
                                nc.sync.dma_start(mask_sb[:], maskTb.ap())
                                nc.sync.dma_start(id_sb[:], idb.ap())
                                for nm, t in (("Aq", ropeAq), ("Bq", ropeBq),
                                              ("Ak", ropeAk), ("Bk", ropeBk)):
                                    nc.sync.dma_start(ropes[nm][:], t.ap())

                        def emit_raw(n):
                            for mt in range(4):
                                rh = raw_h[mt]
                                if n > 0:
                                    nc.vector.tensor_copy(
                                        rh[:, 1:4], rh[:, 513:516])
                                nc.vector.tensor_copy(rh[:, 4:516],
                                                      psums[mt][:])

                        def emit_canon(n):
                            lo = 512 * n
                            for mt in range(4):
                                rh = raw_h[mt]
                                c = cn[mt]
                                nc.vector.tensor_scalar_mul(
                                    c[:, lo:lo + 512], rh[:, 4:516],
                                    s0_sb[mt][:])
                                for k in range(1, K_CONV):
                                    nc.vector.scalar_tensor_tensor(
                                        c[:, lo:lo + 512],
                                        rh[:, 4 - k:516 - k],
                                        cw_sb[mt][:, k:k + 1],
                                        c[:, lo:lo + 512],
                                        ALU.mult, ALU.add)
                            # squares for rmsnorm (q0, q1, k)
                            for mt in range(3):
                                sq = bw.tile([128, 512], BF16, tag="sqr",
                                             bufs=3, name="sq")
                                nc.vector.tensor_mul(
                                    sq[:], cn[mt][:, lo:lo + 512],
                                    cn[mt][:, lo:lo + 512])
                                sqs[(n, mt)] = sq

                        def emit_norm(n):
                            for mt in range(3):
                                sp = spp.tile([1, 512], F32, tag="ssq")
                                nc.tensor.matmul(sp[:], ones_col[:],
                                                 sqs.pop((n, mt))[:],
                                                 start=True, stop=True)
                                srt = bw.tile([1, 512], F32, tag="srt",
                                              bufs=2, name="srt")
                                nc.scalar.activation(srt[:], sp[:], AF.Sqrt,
                                                     bias=eps_sb[:],
                                                     scale=1.0 / DH)
                                rn = bw.tile([1, 512], F32, tag=f"rn{mt}",
                                             bufs=3, name=f"rn{mt}")
                                nc.vector.reciprocal_approx_fast(
                                    rn[:], srt[:])
                                rns[(n, mt)] = rn
                            # V transpose blocks for this chunk
                            for t in range(4):
                                i = 4 * n + t
                                nc.sync.dma_start_transpose(
                                    vT[b][:, 128 * i:128 * (i + 1)],
                                    cn[3][:, 128 * i:128 * (i + 1)])

                        def emit_bc(n):
                            # rstd broadcast for chunk n (PE + ACT only;
                            # emitted inside the PE-dense chunk stream)
                            for mt in range(3):
                                bp = bps.tile([128, 512], F32, tag="bcp")
                                nc.tensor.matmul(
                                    bp[:], ones_row[:],
                                    rns.pop((n, mt))[:],
                                    start=True, stop=True)
                                bcb = bw.tile([128, 512], BF16, tag="bcs",
                                              bufs=4, name="bcs")
                                nc.scalar.copy(bcb[:], bp[:])
                                bcb_store[(mt, n)] = bcb

                        def emit_rope1(n):
                            # chunk-wise rope + in-place rstd scale so
                            # roped is complete when the pipeline drains
                            lo = 512 * n
                            for mt in (2, 0, 1):
                                is_q = mt < 2
                                x = cn[mt]
                                A_ = ropes["Aq"] if is_q else ropes["Ak"]
                                B_ = ropes["Bq"] if is_q else ropes["Bk"]
                                sh = bw.tile([128, 512], BF16, tag="shift",
                                             bufs=3, name="sh")
                                nc.gpsimd.dma_start(sh[0:64, :],
                                                    x[64:128, lo:lo + 512])
                                nc.gpsimd.dma_start(sh[64:128, :],
                                                    x[0:64, lo:lo + 512])
                                nc.vector.tensor_mul(sh[:], sh[:],
                                                     B_[:, lo:lo + 512])
                                ro = roped[(b, mt)]
                                nc.vector.tensor_mul(ro[:, lo:lo + 512],
                                                     x[:, lo:lo + 512],
                                                     A_[:, lo:lo + 512])
                                nc.vector.tensor_add(ro[:, lo:lo + 512],
                                                     ro[:, lo:lo + 512],
                                                     sh[:])
                                nc.vector.tensor_mul(
                                    ro[:, lo:lo + 512],
                                    ro[:, lo:lo + 512],
                                    bcb_store.pop((mt, n))[:])

                        sqs = {}
                        rns = {}
                        bcb_store = {}
                        for n in range(NCB + 2):
                            if n < NCB:
                                emit_chunk_mms(n)
                            if 1 <= n <= NCB:
                                emit_norm(n - 1)
                            if n >= 2:
                                emit_bc(n - 2)
                            if n < NCB:
                                emit_raw(n)
                                emit_canon(n)
                            if n >= 2:
                                emit_rope1(n - 2)

            # ============ attention + head-split all-to-all ============
            wpool_ctx = tc.tile_pool(name="wpool", bufs=1)
            wpool = wpool_ctx.__enter__()
            # Wo resident prefetch (needed only for the output projection)
            wo_sb = wpool.tile([128, 16 * D], BF16, tag="wosb")
            wov = wo_sb[:].rearrange("p (g o) -> p g o", o=D)
            for gg in range(4):
                nc.sync.dma_start(
                    wov[:, 4 * gg:4 * (gg + 1), :],
                    woT.ap()[512 * gg:512 * (gg + 1), :]
                    .rearrange("(g p) o -> p g o", p=128))

            a2a_in = {}
            a2a_out = {}
            oin = {}
            for h in range(2):
                a2a_in[h] = dram.tile([1024, 512], BF16, tag=f"a2ain{h}",
                                      name=f"a2a_in{h}")
                a2a_out[h] = dram.tile([1024, 512], BF16, tag=f"a2aout{h}",
                                       name=f"a2a_out{h}")
                oin[h] = wpool.tile([128, 8 * 512], BF16, tag=f"oin{h}",
                                    name=f"oin{h}")

            with tc.tile_pool(name="scps", bufs=2, space="PSUM") as scps, \
                 tc.tile_pool(name="pvps", bufs=2, space="PSUM") as pvps, \
                 tc.tile_pool(name="dnps", bufs=1, space="PSUM") as dnps, \
                 tc.tile_pool(name="bcps", bufs=1, space="PSUM") as bcps:
                pending = [None]

                def flush_tail():
                    if pending[0] is None:
                        return
                    pv, dn, h, b, j = pending[0]
                    pending[0] = None
                    rec = atop.tile([1, 512], F32, tag="rec",
                                     bufs=2, name="rec")
                    nc.vector.reciprocal_approx_fast(rec[:], dn[:])
                    bc = bcps.tile([128, 512], F32, tag="bc", name="bc")
                    nc.tensor.matmul(bc[:], ones_row[:], rec[:],
                                     start=True, stop=True)
                    bcb = atop.tile([128, 512], BF16, tag="bcbn",
                                     bufs=2, name="bcb")
                    nc.vector.tensor_copy(bcb[:], bc[:])
                    nrm = atop.tile([128, 512], BF16, tag="nrm",
                                     bufs=2, name="nrm")
                    nc.vector.tensor_mul(nrm[:], pv[:], bcb[:])
                    nc.sync.dma_start(
                        a2a_in[h][128 * (4 * b + j):
                                  128 * (4 * b + j + 1), :],
                        nrm[:])

                for h in range(2):
                    for b in range(B):
                        KT = roped[(b, 2)]
                        QT = roped[(b, h)]
                        vt = vT[b]
                        for j in range(NCB):
                            pv = pvps.tile([128, 512], F32, tag="pv",
                                           name="pv")
                            dn = dnps.tile([1, 512], F32, tag="dn",
                                           name="dn")
                            acc = atop.tile([128, 512], BF16, tag="acc",
                                             bufs=2, name="acc")
                            nprs = 2 * j + 2   # pairs of Sk blocks
                            pts = [None] * nprs
                            offp = [None] * nprs

                            def emit_qk(pr):
                                ps = scps.tile([128, 1024], F32, tag="sc",
                                               name="ps")
                                pt = atop.tile([128, 1024], BF16, tag="p",
                                                bufs=4, name="pt")
                                offs = []
                                for half in range(2):
                                    i = 2 * pr + half
                                    r = i - 4 * j
                                    off = 128 * max(r, 0)
                                    offs.append(off)
                                    base = 512 * half
                                    diag = (r >= 0)
                                    nc.tensor.matmul(
                                        ps[:, base + off:base + 512],
                                        KT[:, 128 * i:128 * (i + 1)],
                                        QT[:, 512 * j + off:512 * (j + 1)],
                                        start=True, stop=not diag)
                                    if diag:
                                        nc.tensor.matmul(
                                            ps[:, base + off:
                                               base + off + 128],
                                            mask_sb[:], id_sb[:],
                                            start=False, stop=True,
                                            skip_group_check=True)
                                if offs[1] > 0:
                                    nc.tensor.matmul(
                                        ps[:, 512:512 + offs[1]],
                                        negrow[:], onesb[:, 0:offs[1]],
                                        start=True, stop=True)
                                nc.scalar.activation(
                                    pt[:, offs[0]:1024],
                                    ps[:, offs[0]:1024], AF.Exp)
                                pts[pr] = pt
                                offp[pr] = offs
                                # denominator partials on DVE
                                if pr == 0:
                                    nc.vector.tensor_copy(
                                        acc[:], pt[:, 0:512])
                                else:
                                    nc.vector.tensor_add(
                                        acc[:, offs[0]:512],
                                        acc[:, offs[0]:512],
                                        pt[:, offs[0]:512])
                                nc.vector.tensor_add(
                                    acc[:, offs[1]:512],
                                    acc[:, offs[1]:512],
                                    pt[:, 512 + offs[1]:1024])

                            def emit_pv(pr):
                                pt = pts[pr]
                                offs = offp[pr]
                                for half in range(2):
                                    i = 2 * pr + half
                                    off = offs[half]
                                    first = (i == 0)
                                    last = (i == 4 * j + 3)
                                    base = 512 * half
                                    nc.tensor.matmul(
                                        pv[:, off:512],
                                        vt[:, 128 * i:128 * (i + 1)],
                                        pt[:, base + off:base + 512],
                                        start=first, stop=last,
                                        skip_group_check=True)

                            for pr in range(nprs):
                                emit_qk(pr)
                                if pr == min(1, nprs - 1):
                                    flush_tail()
                                if pr >= 1:
                                    emit_pv(pr - 1)
                            emit_pv(nprs - 1)
                            nc.tensor.matmul(dn[:], ones_col[:], acc[:],
                                             start=True, stop=True)
                            pending[0] = (pv, dn, h, b, j)
                    flush_tail()
                    nc.gpsimd.collective_compute(
                        "AllToAll", ALU.bypass,
                        replica_groups=[list(range(N_CORES))],
                        ins=[a2a_in[h].opt()], outs=[a2a_out[h].opt()],
                        cc_dim="Partition")
                    # gpsimd queue: keeps the sync queue free for the
                    # h=1 staging DMAs while the collective runs; per-s
                    # splits let the out projection start on block 0
                    # as soon as it lands
                    oiv = oin[h][:].rearrange("p (s t) -> p s t", t=512)
                    for s_ in range(8):
                        nc.gpsimd.dma_start(
                            oiv[:, s_:s_ + 1, :],
                            a2a_out[h][:].rearrange("(s p) t -> p s t",
                                                    p=128)[:, s_:s_ + 1, :])

            # ====================== out projection ====================
            # pass A: h=0 partial sums for all (n, mp) -> SBUF (runs during
            # the second all-to-all); pass B: h=1 partials + DVE combine.
            ovs = {h: oin[h][:].rearrange("p (s t) -> p s t", t=512)
                   for h in range(2)}
            with tc.tile_pool(name="opool", bufs=1) as opool, \
                 tc.tile_pool(name="ops", bufs=2, space="PSUM") as ops:
                ph0 = {}
                for n in range(4):
                    for mp in range(4):
                        pso = ops.tile([128, 512], F32, tag=f"oa{mp}",
                                       name=f"oa{mp}")
                        for s in range(8):
                            nc.tensor.matmul(
                                pso[:],
                                ovs[0][:, s, 128 * mp:128 * (mp + 1)],
                                wov[:, 2 * s, 512 * n:512 * (n + 1)],
                                start=(s == 0), stop=(s == 7))
                        pt0 = opool.tile([128, 512], F32, tag="ph0",
                                         bufs=16, name="pt0")
                        nc.scalar.copy(pt0[:], pso[:])
                        ph0[(n, mp)] = pt0
                for n in range(4):
                    for mp in range(4):
                        pso = ops.tile([128, 512], F32, tag=f"oa{mp}",
                                       name=f"ob{mp}")
                        for s in range(8):
                            nc.tensor.matmul(
                                pso[:],
                                ovs[1][:, s, 128 * mp:128 * (mp + 1)],
                                wov[:, 2 * s + 1, 512 * n:512 * (n + 1)],
                                start=(s == 0), stop=(s == 7))
                        os_t = opool.tile([128, 512], F32, tag="osb",
                                          bufs=4, name="os_t")
                        nc.vector.tensor_add(os_t[:], pso[:],
                                             ph0[(n, mp)][:])
                        nc.sync.dma_start(
                            out.ap()[128 * mp:128 * (mp + 1),
                                     512 * n:512 * (n + 1)],
                            os_t[:])
            wpool_ctx.__exit__(None, None, None)
            atop_ctx.__exit__(None, None, None)

    nc.compile()
    return nc


_NC_CACHE = None


def _get_nc():
    global _NC_CACHE
    if _NC_CACHE is None:
        _NC_CACHE = _build()
    return _NC_CACHE


def _host_prep(inputs):
    hs = np.asarray(inputs["hidden_states"], dtype=np.float32)
    Wq = np.asarray(inputs["Wq"], dtype=np.float32)
    Wk = np.asarray(inputs["Wk"], dtype=np.float32)
    Wv = np.asarray(inputs["Wv"], dtype=np.float32)
    Wo = np.asarray(inputs["Wo"], dtype=np.float32)
    cqw = np.asarray(inputs["canon_q_w"], dtype=np.float32)
    ckw = np.asarray(inputs["canon_k_w"], dtype=np.float32)
    cvw = np.asarray(inputs["canon_v_w"], dtype=np.float32)
    qnw = np.asarray(inputs["q_norm_w"], dtype=np.float32)
    knw = np.asarray(inputs["k_norm_w"], dtype=np.float32)

    bf = ml_dtypes.bfloat16
    hsT = np.ascontiguousarray(
        np.concatenate([hs[0].T, hs[1].T], axis=1)).astype(bf)
    WqT, WkT, WvT = Wq.T, Wk.T, Wv.T
    woT = np.ascontiguousarray(Wo.T).astype(bf)

    inv_freq = 1.0 / (10000.0 ** (np.arange(0, DH, 2, dtype=np.float64) / DH))
    freqs = np.arange(S, dtype=np.float64)[:, None] * inv_freq
    emb = np.concatenate([freqs, freqs], axis=-1)
    cosT, sinT = np.cos(emb).T, np.sin(emb).T

    def make_rope(normw, scale):
        A = cosT * normw[:, None] * scale
        wswap = normw[(np.arange(DH) + 64) % DH]
        sign = np.where(np.arange(DH) < 64, -1.0, 1.0)
        Bc = sinT * wswap[:, None] * sign[:, None] * scale
        return (np.ascontiguousarray(A).astype(bf),
                np.ascontiguousarray(Bc).astype(bf))

    Aq, Bq = make_rope(qnw, SCALE)
    Ak, Bk = make_rope(knw, 1.0)

    p = np.arange(128)[:, None]
    f = np.arange(128)[None, :]
    maskd = np.where(p <= f, 0.0, NEG).astype(np.float32)
    maskTb = np.ascontiguousarray(maskd.T).astype(bf)
    idb = np.eye(128, dtype=np.float32).astype(bf)

    in_maps = []
    for r in range(N_CORES):
        wTc = np.ascontiguousarray(np.concatenate(
            [WqT[:, 256 * r:256 * r + 256],
             WkT[:, 128 * r:128 * r + 128],
             WvT[:, 128 * r:128 * r + 128]], axis=1)).astype(bf)
        cwc = np.ascontiguousarray(np.concatenate(
            [cqw[256 * r:256 * r + 256],
             ckw[128 * r:128 * r + 128],
             cvw[128 * r:128 * r + 128]], axis=0)).astype(np.float32)
        in_maps.append({
            "hsT": hsT, "wT": wTc, "woT": woT, "cw": cwc,
            "ropeAq": Aq, "ropeBq": Bq, "ropeAk": Ak, "ropeBk": Bk,
            "maskTb": maskTb, "idb": idb,
        })
    return in_maps


def kernel(**inputs):
    nc = _get_nc()
    in_maps = _host_prep(inputs)
    res = run_bass_kernel_spmd(nc, in_maps, core_ids=list(range(N_CORES)))
    full = np.empty((B, S, D), np.float32)
    for r in range(N_CORES):
        full[r // 4, 512 * (r % 4):512 * (r % 4 + 1), :] = res.results[r]["out"]
    return full
